# revision 6
# baseline (speedup 1.0000x reference)
"""LIF spiking network (2-layer, subtract-reset) on 8 Trainium2 NeuronCores.

Strategy: data-parallel over batch B=512 -> 64 batch elements per core.
The T=512 recurrence runs locally per core (no cross-device comm).

Per-core layouts (chosen so no on-device transposes are needed):
  xT      (8, T*64)     [i, (t,b)]   host-pretransposed input slice
  w1t     (8, 256)      W1.T
  w2t     (256, 128)    W2.T
  mem1    (128, 2*64)   [p, (c,b)]   neuron h1 = c*128+p
  mem2/spk2 live directly in the output staging tiles (128, TT*64) [h2,(t,b)]
  out_mem/out_spk DRAM (128, T, 64) [h2, t, b]; host transposes to (T,B,H2).

Per step: 2 fp32 matmuls (layer1, K=8) -> PSUM; DVE: mem1 = mem1*b1 + cur1
(fused scalar_tensor_tensor), mem1 -= spk1_prev, spk1 = (mem1 > 1); 2 fp32
matmuls (layer2, K=128 each, PSUM-accumulated); same DVE update for mem2
written straight into the DMA staging buffer. Staging double-buffered,
DMA'd out per TT-step block.

The anomaly head (tiny: mean over T, dot with (1,128) weights, sigmoid) is
computed on host from the device-produced spike stack.
"""

import numpy as np

T = 512
B_FULL = 512
N_CORES = 8
B = B_FULL // N_CORES  # 64 per core
NIN = 8
H1 = 256
H2 = 128
TT = 32  # timesteps per staging block
BETA1, BETA2, TH = 0.95, 0.9, 1.0

_CACHE = {}


def _build(n_steps=T):
    import concourse.bacc as bacc
    import concourse.mybir as mybir
    from concourse.tile import TileContext
    from concourse.alu_op_type import AluOpType as alu

    f32 = mybir.dt.float32
    nblk = n_steps // TT

    nc = bacc.Bacc("TRN2", target_bir_lowering=False, debug=False)
    xT = nc.dram_tensor("xt", (NIN, n_steps * B), f32, kind="ExternalInput")
    w1t = nc.dram_tensor("w1t", (NIN, H1), f32, kind="ExternalInput")
    w2t = nc.dram_tensor("w2t", (H1, H2), f32, kind="ExternalInput")
    out_mem = nc.dram_tensor("out_mem", (H2, n_steps, B), f32, kind="ExternalOutput")
    out_spk = nc.dram_tensor("out_spk", (H2, n_steps, B), f32, kind="ExternalOutput")

    with TileContext(nc) as tc:
        with (
            tc.tile_pool(name="const", bufs=1) as cpool,
            tc.tile_pool(name="state", bufs=1) as spool,
            tc.tile_pool(name="xin", bufs=2) as xpool,
            tc.tile_pool(name="stmem", bufs=3) as smpool,
            tc.tile_pool(name="stspk", bufs=3) as sspool,
            tc.tile_pool(name="ps1", bufs=3, space="PSUM") as ps1pool,
            tc.tile_pool(name="ps2", bufs=3, space="PSUM") as ps2pool,
        ):
            w1sb = cpool.tile([NIN, H1], f32)
            nc.sync.dma_start(out=w1sb[:], in_=w1t[:])
            w2sb = cpool.tile([128, 256], f32)
            nc.sync.dma_start(out=w2sb[:, 0:128], in_=w2t[0:128, :])
            nc.sync.dma_start(out=w2sb[:, 128:256], in_=w2t[128:256, :])

            mem1 = spool.tile([128, 2 * B], f32)
            nc.vector.memset(mem1[:], 0.0)
            spk1 = spool.tile([128, 2 * 2 * B], f32)  # two rolling slots

            prev_sm = None
            prev_ss = None
            for blk in range(nblk):
                xblk = xpool.tile([NIN, TT * B], f32)
                nc.sync.dma_start(
                    out=xblk[:], in_=xT[:, blk * TT * B : (blk + 1) * TT * B]
                )
                sm = smpool.tile([128, TT * B], f32)
                ss = sspool.tile([128, TT * B], f32)
                for k in range(TT):
                    t = blk * TT + k
                    sl = (t % 2) * 2 * B
                    pl = ((t - 1) % 2) * 2 * B
                    ps1 = ps1pool.tile([128, 2 * B], f32)
                    rhs_x = xblk[:, k * B : (k + 1) * B]
                    nc.tensor.matmul(
                        ps1[:, 0:B], w1sb[:, 0:128], rhs_x, start=True, stop=True
                    )
                    nc.tensor.matmul(
                        ps1[:, B : 2 * B],
                        w1sb[:, 128:256],
                        rhs_x,
                        start=True,
                        stop=True,
                    )
                    # mem1 = mem1*beta1 + cur1
                    nc.vector.scalar_tensor_tensor(
                        mem1[:], mem1[:], BETA1, ps1[:], alu.mult, alu.add
                    )
                    if t > 0:
                        nc.vector.tensor_tensor(
                            mem1[:], mem1[:], spk1[:, pl : pl + 2 * B], alu.subtract
                        )
                    nc.vector.tensor_scalar(
                        spk1[:, sl : sl + 2 * B], mem1[:], TH, None, alu.is_gt
                    )
                    ps2 = ps2pool.tile([128, B], f32)
                    nc.tensor.matmul(
                        ps2[:],
                        w2sb[:, 0:128],
                        spk1[:, sl : sl + B],
                        start=True,
                        stop=False,
                    )
                    nc.tensor.matmul(
                        ps2[:],
                        w2sb[:, 128:256],
                        spk1[:, sl + B : sl + 2 * B],
                        start=False,
                        stop=True,
                    )
                    dst = sm[:, k * B : (k + 1) * B]
                    if t == 0:
                        nc.vector.tensor_copy(dst, ps2[:])
                    else:
                        m2prev = (
                            sm[:, (k - 1) * B : k * B]
                            if k > 0
                            else prev_sm[:, (TT - 1) * B : TT * B]
                        )
                        nc.vector.scalar_tensor_tensor(
                            dst, m2prev, BETA2, ps2[:], alu.mult, alu.add
                        )
                        s2prev = (
                            ss[:, (k - 1) * B : k * B]
                            if k > 0
                            else prev_ss[:, (TT - 1) * B : TT * B]
                        )
                        nc.vector.tensor_tensor(dst, dst, s2prev, alu.subtract)
                    nc.vector.tensor_scalar(
                        ss[:, k * B : (k + 1) * B], dst, TH, None, alu.is_gt
                    )
                nc.sync.dma_start(
                    out=out_mem[:, blk * TT : (blk + 1) * TT, :],
                    in_=sm[:].rearrange("p (t b) -> p t b", b=B),
                )
                nc.sync.dma_start(
                    out=out_spk[:, blk * TT : (blk + 1) * TT, :],
                    in_=ss[:].rearrange("p (t b) -> p t b", b=B),
                )
                prev_sm, prev_ss = sm, ss
    nc.compile()
    return nc


def _get_nc(n_steps=T):
    key = ("v4", n_steps)
    if key not in _CACHE:
        _CACHE[key] = _build(n_steps)
    return _CACHE[key]


def run_on_device(spike_input, W1, W2, n_steps=T, trace=False):
    """Run the bass kernel on 8 cores. Returns (mem2, spk2) as
    (n_steps, B_FULL, H2) float32 arrays plus the BassKernelResults."""
    from concourse.bass_utils import run_bass_kernel_spmd

    nc = _get_nc(n_steps)
    w1t_h = np.ascontiguousarray(np.asarray(W1, np.float32).T)
    w2t_h = np.ascontiguousarray(np.asarray(W2, np.float32).T)
    x = np.asarray(spike_input, np.float32)[:n_steps]
    in_maps = []
    for c in range(N_CORES):
        b0 = c * B
        xT_h = np.ascontiguousarray(
            x[:, b0 : b0 + B, :].transpose(2, 0, 1).reshape(NIN, n_steps * B)
        )
        in_maps.append({"xt": xT_h, "w1t": w1t_h, "w2t": w2t_h})
    res = run_bass_kernel_spmd(
        nc, in_maps, core_ids=list(range(N_CORES)), trace=trace
    )
    mem = np.empty((n_steps, B_FULL, H2), np.float32)
    spk = np.empty((n_steps, B_FULL, H2), np.float32)
    for c in range(N_CORES):
        b0 = c * B
        mem[:, b0 : b0 + B, :] = res.results[c]["out_mem"].transpose(1, 2, 0)
        spk[:, b0 : b0 + B, :] = res.results[c]["out_spk"].transpose(1, 2, 0)
    return spk, mem, res


def kernel(spike_input, W1, W2, W_out, b_out):
    spk_stack, mem_stack, _ = run_on_device(spike_input, W1, W2)
    W_out = np.asarray(W_out, np.float32)
    b_out = np.asarray(b_out, np.float32)
    mean_firing = spk_stack.mean(axis=0, dtype=np.float32)  # (B, H2)
    logits = mean_firing @ W_out.T + b_out  # (B, 1)
    anomaly_prob = (1.0 / (1.0 + np.exp(-logits.astype(np.float32)))).squeeze(-1)
    return spk_stack, mem_stack, anomaly_prob.astype(np.float32)


# revision 7
# speedup vs baseline: 1.2158x; 1.2158x over previous
"""LIF spiking network (2-layer, subtract-reset) on 8 Trainium2 NeuronCores.

Strategy: data-parallel over batch B=512 -> 64 batch elements per core.
The T=512 recurrence runs locally per core (no cross-device comm).

Per-core layouts (chosen so no on-device transposes are needed):
  xT      (8, T*64)     [i, (t,b)]   host-pretransposed input slice
  w1t     (8, 256)      W1.T
  w2t     (256, 128)    W2.T
  mem1    (128, 2*64)   [p, (c,b)]   neuron h1 = c*128+p
  mem2/spk2 live directly in the output staging tiles (128, TT*64) [h2,(t,b)]
  out_mem/out_spk DRAM (128, T, 64) [h2, t, b]; host transposes to (T,B,H2).

Per step: 2 fp32 matmuls (layer1, K=8) -> PSUM; DVE: mem1 = mem1*b1 + cur1
(fused scalar_tensor_tensor), mem1 -= spk1_prev, spk1 = (mem1 > 1); 2 fp32
matmuls (layer2, K=128 each, PSUM-accumulated); same DVE update for mem2
written straight into the DMA staging buffer. Staging double-buffered,
DMA'd out per TT-step block.

The anomaly head (tiny: mean over T, dot with (1,128) weights, sigmoid) is
computed on host from the device-produced spike stack.
"""

import numpy as np

T = 512
B_FULL = 512
N_CORES = 8
B = B_FULL // N_CORES  # 64 per core
NIN = 8
H1 = 256
H2 = 128
TT = 32  # timesteps per staging block
BETA1, BETA2, TH = 0.95, 0.9, 1.0

_CACHE = {}


def _build(n_steps=T):
    import concourse.bacc as bacc
    import concourse.mybir as mybir
    from concourse.tile import TileContext
    from concourse.alu_op_type import AluOpType as alu

    f32 = mybir.dt.float32
    nblk = n_steps // TT

    nc = bacc.Bacc("TRN2", target_bir_lowering=False, debug=False)
    xT = nc.dram_tensor("xt", (NIN, n_steps * B), f32, kind="ExternalInput")
    w1t = nc.dram_tensor("w1t", (NIN, H1), f32, kind="ExternalInput")
    w2t = nc.dram_tensor("w2t", (H1, H2), f32, kind="ExternalInput")
    out_mem = nc.dram_tensor("out_mem", (H2, n_steps, B), f32, kind="ExternalOutput")
    out_spk = nc.dram_tensor("out_spk", (H2, n_steps, B), f32, kind="ExternalOutput")

    with TileContext(nc) as tc:
        with (
            tc.tile_pool(name="const", bufs=1) as cpool,
            tc.tile_pool(name="state", bufs=1) as spool,
            tc.tile_pool(name="xin", bufs=2) as xpool,
            tc.tile_pool(name="stmem", bufs=3) as smpool,
            tc.tile_pool(name="stspk", bufs=3) as sspool,
            tc.tile_pool(name="ps1", bufs=3, space="PSUM") as ps1pool,
            tc.tile_pool(name="ps2", bufs=3, space="PSUM") as ps2pool,
            tc.tile_pool(name="cur1", bufs=3) as c1pool,
            tc.tile_pool(name="cur2", bufs=3) as c2pool,
        ):
            w1sb = cpool.tile([NIN, H1], f32)
            nc.sync.dma_start(out=w1sb[:], in_=w1t[:])
            w2sb = cpool.tile([128, 256], f32)
            nc.sync.dma_start(out=w2sb[:, 0:128], in_=w2t[0:128, :])
            nc.sync.dma_start(out=w2sb[:, 128:256], in_=w2t[128:256, :])

            mem1 = spool.tile([128, 2 * B], f32)
            nc.vector.memset(mem1[:], 0.0)
            spk1 = spool.tile([128, 2 * 2 * B], f32)  # two rolling slots

            prev_sm = None
            prev_ss = None
            for blk in range(nblk):
                xblk = xpool.tile([NIN, TT * B], f32)
                nc.sync.dma_start(
                    out=xblk[:], in_=xT[:, blk * TT * B : (blk + 1) * TT * B]
                )
                sm = smpool.tile([128, TT * B], f32)
                ss = sspool.tile([128, TT * B], f32)
                for k in range(TT):
                    t = blk * TT + k
                    sl = (t % 2) * 2 * B
                    pl = ((t - 1) % 2) * 2 * B
                    ps1 = ps1pool.tile([128, 2 * B], f32)
                    rhs_x = xblk[:, k * B : (k + 1) * B]
                    nc.tensor.matmul(
                        ps1[:, 0:B], w1sb[:, 0:128], rhs_x, start=True, stop=True
                    )
                    nc.tensor.matmul(
                        ps1[:, B : 2 * B],
                        w1sb[:, 128:256],
                        rhs_x,
                        start=True,
                        stop=True,
                    )
                    cur1 = c1pool.tile([128, 2 * B], f32)
                    nc.scalar.copy(cur1[:], ps1[:])  # exact; moves PSUM read off DVE
                    # mem1 = mem1*beta1 + cur1
                    nc.vector.scalar_tensor_tensor(
                        mem1[:], mem1[:], BETA1, cur1[:], alu.mult, alu.add
                    )
                    if t > 0:
                        nc.vector.tensor_tensor(
                            mem1[:], mem1[:], spk1[:, pl : pl + 2 * B], alu.subtract
                        )
                    nc.vector.tensor_scalar(
                        spk1[:, sl : sl + 2 * B], mem1[:], TH, None, alu.is_gt
                    )
                    ps2 = ps2pool.tile([128, B], f32)
                    nc.tensor.matmul(
                        ps2[:],
                        w2sb[:, 0:128],
                        spk1[:, sl : sl + B],
                        start=True,
                        stop=False,
                    )
                    nc.tensor.matmul(
                        ps2[:],
                        w2sb[:, 128:256],
                        spk1[:, sl + B : sl + 2 * B],
                        start=False,
                        stop=True,
                    )
                    cur2 = c2pool.tile([128, B], f32)
                    nc.scalar.copy(cur2[:], ps2[:])
                    dst = sm[:, k * B : (k + 1) * B]
                    if t == 0:
                        nc.vector.tensor_copy(dst, cur2[:])
                    else:
                        m2prev = (
                            sm[:, (k - 1) * B : k * B]
                            if k > 0
                            else prev_sm[:, (TT - 1) * B : TT * B]
                        )
                        nc.vector.scalar_tensor_tensor(
                            dst, m2prev, BETA2, cur2[:], alu.mult, alu.add
                        )
                        s2prev = (
                            ss[:, (k - 1) * B : k * B]
                            if k > 0
                            else prev_ss[:, (TT - 1) * B : TT * B]
                        )
                        nc.vector.tensor_tensor(dst, dst, s2prev, alu.subtract)
                    nc.vector.tensor_scalar(
                        ss[:, k * B : (k + 1) * B], dst, TH, None, alu.is_gt
                    )
                nc.sync.dma_start(
                    out=out_mem[:, blk * TT : (blk + 1) * TT, :],
                    in_=sm[:].rearrange("p (t b) -> p t b", b=B),
                )
                nc.sync.dma_start(
                    out=out_spk[:, blk * TT : (blk + 1) * TT, :],
                    in_=ss[:].rearrange("p (t b) -> p t b", b=B),
                )
                prev_sm, prev_ss = sm, ss
    nc.compile()
    return nc


def _get_nc(n_steps=T):
    key = ("v5", n_steps)
    if key not in _CACHE:
        _CACHE[key] = _build(n_steps)
    return _CACHE[key]


def run_on_device(spike_input, W1, W2, n_steps=T, trace=False):
    """Run the bass kernel on 8 cores. Returns (mem2, spk2) as
    (n_steps, B_FULL, H2) float32 arrays plus the BassKernelResults."""
    from concourse.bass_utils import run_bass_kernel_spmd

    nc = _get_nc(n_steps)
    w1t_h = np.ascontiguousarray(np.asarray(W1, np.float32).T)
    w2t_h = np.ascontiguousarray(np.asarray(W2, np.float32).T)
    x = np.asarray(spike_input, np.float32)[:n_steps]
    in_maps = []
    for c in range(N_CORES):
        b0 = c * B
        xT_h = np.ascontiguousarray(
            x[:, b0 : b0 + B, :].transpose(2, 0, 1).reshape(NIN, n_steps * B)
        )
        in_maps.append({"xt": xT_h, "w1t": w1t_h, "w2t": w2t_h})
    res = run_bass_kernel_spmd(
        nc, in_maps, core_ids=list(range(N_CORES)), trace=trace
    )
    mem = np.empty((n_steps, B_FULL, H2), np.float32)
    spk = np.empty((n_steps, B_FULL, H2), np.float32)
    for c in range(N_CORES):
        b0 = c * B
        mem[:, b0 : b0 + B, :] = res.results[c]["out_mem"].transpose(1, 2, 0)
        spk[:, b0 : b0 + B, :] = res.results[c]["out_spk"].transpose(1, 2, 0)
    return spk, mem, res


def kernel(spike_input, W1, W2, W_out, b_out):
    spk_stack, mem_stack, _ = run_on_device(spike_input, W1, W2)
    W_out = np.asarray(W_out, np.float32)
    b_out = np.asarray(b_out, np.float32)
    mean_firing = spk_stack.mean(axis=0, dtype=np.float32)  # (B, H2)
    logits = mean_firing @ W_out.T + b_out  # (B, 1)
    anomaly_prob = (1.0 / (1.0 + np.exp(-logits.astype(np.float32)))).squeeze(-1)
    return spk_stack, mem_stack, anomaly_prob.astype(np.float32)
